# revision 7
# baseline (speedup 1.0000x reference)
"""Trainium2 Bass kernel for nn_CausalPredictor_46462956208724.

Math: the reference computes
    wy = xm @ Wy_w.T + Wy_b            [L, 1]
    wz = dic_z @ Wz_w.T + Wz_b         [1, 1]
    attention = softmax(wy @ wz.T, axis=1)   # axis of size 1 -> exactly 1.0
    z = (attention * prior) @ dic_z    [L, C]
Softmax over a size-1 axis is exactly 1.0 in fp32 (exp(0)/exp(0)), so
    z[l, :] = prior[0] * dic[1, 0, :]   for every row l.
The output is a broadcast of one scaled 1024-float row to 131072 rows —
a pure HBM-write problem (512 MB of output).

Sharding: pure data parallel over rows. 8 cores x 16384 rows each; the
tiny row + prior are replicated. Each core scales the row on-chip,
replicates it across 128 SBUF partitions x RP repeats, and stores its
64 MB output shard with DMA(s) whose SBUF-side access pattern carries a
stride-0 repeat dim, so one small SBUF tile feeds the whole shard.
"""

import sys

for _p in (
    "/root/.axon_site",
    "/root/.axon_site/_ro/trn_rl_repo",
    "/root/.axon_site/_ro/pypackages",
    "/opt/trn_rl_repo",
):
    if _p not in sys.path:
        sys.path.append(_p)

import numpy as np

L = 131072
C = 1024
N_CORES = 8
SHARD = L // N_CORES          # 16384 rows per core
P = 128                       # SBUF partitions
RPP = SHARD // P              # 128 output rows per partition in the shard

# Strategy: "one_dma" = single 64 MB DMA, stride-0 repeat over a small
# tile; "blocks" = NBLK big-tile DMAs without stride-0 source.
STRATEGY = "one_dma"
RP = 4                        # physically materialized repeats per partition
NBLK = 8                      # DMA count for "blocks" strategy

_CACHE = {}


def _build_bass():
    import concourse.bacc as bacc
    import concourse.tile as tile
    from concourse import mybir

    f32 = mybir.dt.float32
    # Bacc (not raw Bass): its compile() pipeline splits multi-sem waits
    # into event semaphores — TRN2 allows at most 1 wait per instruction,
    # and walrus rejects the raw IR with "Too many sync wait commands".
    nc = bacc.Bacc(None)
    row_in = nc.declare_dram_parameter("row", [1, C], f32, isOutput=False)
    prior_in = nc.declare_dram_parameter("prior", [1, 1], f32, isOutput=False)
    out = nc.declare_dram_parameter("out", [SHARD, C], f32, isOutput=True)

    with tile.TileContext(nc) as tc:
        with tc.tile_pool(name="pool", bufs=1) as pool:
            # Stride-0 partition dim on the DRAM side: every SBUF
            # partition receives the same row/scalar in one normal
            # 128-partition DMA (a [1, N] DMA would be sprayed across
            # all 16 queues instead).
            col = pool.tile([P, C], f32)
            prb = pool.tile([P, 1], f32)
            nc.sync.dma_start(out=col[:], in_=row_in[:].partition_broadcast(P))
            nc.scalar.dma_start(out=prb[:], in_=prior_in[:].partition_broadcast(P))

            # Same-engine copy so the multiply carries one sem wait and
            # FIFO-orders after it (cheaper than an event-sem split).
            prb2 = pool.tile([P, 1], f32)
            nc.vector.tensor_copy(prb2[:], prb[:])

            if STRATEGY == "one_dma":
                rep = RP
                big = pool.tile([P, rep * C], f32)
                nc.vector.tensor_mul(
                    big[:, 0:C], col[:], prb2[:].broadcast_to([P, C])
                )
                w = C
                while w < rep * C:
                    n = min(w, rep * C - w)
                    nc.vector.tensor_copy(big[:, w : w + n], big[:, 0:n])
                    w += n
                src = big[:].unsqueeze(1).broadcast_to([P, RPP // rep, rep * C])
                dst = out[:].rearrange("(p r) c -> p (r c)", p=P)
                nc.sync.dma_start(out=dst, in_=src)
            else:
                rp = RPP // NBLK
                big = pool.tile([P, rp * C], f32)
                nc.vector.tensor_mul(
                    big[:, 0:C], col[:], prb2[:].broadcast_to([P, C])
                )
                w = C
                while w < rp * C:
                    n = min(w, rp * C - w)
                    nc.vector.tensor_copy(big[:, w : w + n], big[:, 0:n])
                    w += n
                out_v = out[:].rearrange("(b p r) c -> b p (r c)", p=P, r=rp)
                for b in range(NBLK):
                    eng = nc.sync if b % 2 == 0 else nc.scalar
                    eng.dma_start(out=out_v[b], in_=big[:])
    nc.compile()
    return nc


def _get_nc():
    if "nc" not in _CACHE:
        _CACHE["nc"] = _build_bass()
    return _CACHE["nc"]


def kernel(x, xm, Wy_w, Wy_b, Wz_w, Wz_b, dic, prior, **_unused):
    from concourse.bass_utils import run_bass_kernel_spmd

    nc = _get_nc()
    row = np.ascontiguousarray(np.asarray(dic, dtype=np.float32)[1].reshape(1, C))
    pr = np.ascontiguousarray(np.asarray(prior, dtype=np.float32).reshape(1, 1))
    in_maps = [{"row": row, "prior": pr} for _ in range(N_CORES)]
    res = run_bass_kernel_spmd(nc, in_maps, list(range(N_CORES)))
    shards = [res.results[i]["out"] for i in range(N_CORES)]
    full = np.concatenate(shards, axis=0).reshape(L, 1, C)
    return full


# revision 9
# speedup vs baseline: 1.1591x; 1.1591x over previous
"""Trainium2 Bass kernel for nn_CausalPredictor_46462956208724.

Math: the reference computes
    wy = xm @ Wy_w.T + Wy_b            [L, 1]
    wz = dic_z @ Wz_w.T + Wz_b         [1, 1]
    attention = softmax(wy @ wz.T, axis=1)   # axis of size 1 -> exactly 1.0
    z = (attention * prior) @ dic_z    [L, C]
Softmax over a size-1 axis is exactly 1.0 in fp32 (exp(0)/exp(0)), so
    z[l, :] = prior[0] * dic[1, 0, :]   for every row l.
The output is a broadcast of one scaled 1024-float row to 131072 rows —
a pure HBM-write problem (512 MB of output).

Sharding: pure data parallel over rows. 8 cores x 16384 rows each; the
tiny row + prior are replicated. Each core scales the row on-chip,
replicates it across 128 SBUF partitions x RP repeats, and stores its
64 MB output shard with DMA(s) whose SBUF-side access pattern carries a
stride-0 repeat dim, so one small SBUF tile feeds the whole shard.
"""

import sys

for _p in (
    "/root/.axon_site",
    "/root/.axon_site/_ro/trn_rl_repo",
    "/root/.axon_site/_ro/pypackages",
    "/opt/trn_rl_repo",
):
    if _p not in sys.path:
        sys.path.append(_p)

import numpy as np

L = 131072
C = 1024
N_CORES = 8
SHARD = L // N_CORES          # 16384 rows per core
P = 128                       # SBUF partitions
RPP = SHARD // P              # 128 output rows per partition in the shard

# Strategy: "pipelined" = 3 DMAs (2/6/56 MB) that start as soon as their
# source prefix is replicated; "one_dma" = single 64 MB DMA, stride-0
# repeat over a small tile; "blocks" = NBLK big-tile DMAs without
# stride-0 source.
STRATEGY = "pipelined"
RP = 4                        # physically materialized repeats per partition
NBLK = 8                      # DMA count for "blocks" strategy

_CACHE = {}


def _build_bass():
    import concourse.bacc as bacc
    import concourse.tile as tile
    from concourse import mybir

    f32 = mybir.dt.float32
    # Bacc (not raw Bass): its compile() pipeline splits multi-sem waits
    # into event semaphores — TRN2 allows at most 1 wait per instruction,
    # and walrus rejects the raw IR with "Too many sync wait commands".
    nc = bacc.Bacc(None)
    row_in = nc.declare_dram_parameter("row", [1, C], f32, isOutput=False)
    prior_in = nc.declare_dram_parameter("prior", [1, 1], f32, isOutput=False)
    out = nc.declare_dram_parameter("out", [SHARD, C], f32, isOutput=True)

    with tile.TileContext(nc) as tc:
        with tc.tile_pool(name="pool", bufs=1) as pool:
            # Stride-0 partition dim on the DRAM side: every SBUF
            # partition receives the same row/scalar in one normal
            # 128-partition DMA (a [1, N] DMA would be sprayed across
            # all 16 queues instead).
            col = pool.tile([P, C], f32)
            prb = pool.tile([P, 1], f32)
            nc.sync.dma_start(out=col[:], in_=row_in[:].partition_broadcast(P))
            nc.scalar.dma_start(out=prb[:], in_=prior_in[:].partition_broadcast(P))

            # Same-engine copy so the multiply carries one sem wait and
            # FIFO-orders after it (cheaper than an event-sem split).
            prb2 = pool.tile([P, 1], f32)
            nc.vector.tensor_copy(prb2[:], prb[:])

            if STRATEGY == "pipelined":
                # big16 holds the scaled row replicated 16x per
                # partition. Output rows are mapped per-partition
                # contiguous: partition p <-> rows [p*RPP, (p+1)*RPP).
                # Three stores, each gated only on its source prefix:
                #   A: rows r[0:4)    <- big16[:, 0:4C]        (2 MB)
                #   B: rows r[4:16)   <- prefix 4C  x3 stride-0 (6 MB)
                #   C: rows r[16:128) <- prefix 16C x7 stride-0 (56 MB)
                # so the bulk store overlaps the replication copies.
                big = pool.tile([P, 16 * C], f32)
                nc.vector.tensor_mul(
                    big[:, 0:C], col[:], prb2[:].broadcast_to([P, C])
                )
                nc.vector.tensor_copy(big[:, C : 2 * C], big[:, 0:C])
                nc.vector.tensor_copy(big[:, 2 * C : 4 * C], big[:, 0 : 2 * C])
                out_pc = out[:].rearrange("(p r) c -> p r c", p=P)
                nc.sync.dma_start(
                    out=out_pc[:, 0:4, :],
                    in_=big[:, 0 : 4 * C],
                )
                nc.scalar.dma_start(
                    out=out_pc[:, 4:16, :],
                    in_=big[:, 0 : 4 * C].unsqueeze(1).broadcast_to([P, 3, 4 * C]),
                )
                nc.vector.tensor_copy(big[:, 4 * C : 8 * C], big[:, 0 : 4 * C])
                nc.vector.tensor_copy(big[:, 8 * C : 16 * C], big[:, 0 : 8 * C])
                nc.sync.dma_start(
                    out=out_pc[:, 16:128, :],
                    in_=big[:, 0 : 16 * C].unsqueeze(1).broadcast_to([P, 7, 16 * C]),
                )
            elif STRATEGY == "one_dma":
                rep = RP
                big = pool.tile([P, rep * C], f32)
                nc.vector.tensor_mul(
                    big[:, 0:C], col[:], prb2[:].broadcast_to([P, C])
                )
                w = C
                while w < rep * C:
                    n = min(w, rep * C - w)
                    nc.vector.tensor_copy(big[:, w : w + n], big[:, 0:n])
                    w += n
                src = big[:].unsqueeze(1).broadcast_to([P, RPP // rep, rep * C])
                dst = out[:].rearrange("(p r) c -> p (r c)", p=P)
                nc.sync.dma_start(out=dst, in_=src)
            else:
                rp = RPP // NBLK
                big = pool.tile([P, rp * C], f32)
                nc.vector.tensor_mul(
                    big[:, 0:C], col[:], prb2[:].broadcast_to([P, C])
                )
                w = C
                while w < rp * C:
                    n = min(w, rp * C - w)
                    nc.vector.tensor_copy(big[:, w : w + n], big[:, 0:n])
                    w += n
                out_v = out[:].rearrange("(b p r) c -> b p (r c)", p=P, r=rp)
                for b in range(NBLK):
                    eng = nc.sync if b % 2 == 0 else nc.scalar
                    eng.dma_start(out=out_v[b], in_=big[:])
    nc.compile()
    return nc


def _get_nc():
    if "nc" not in _CACHE:
        _CACHE["nc"] = _build_bass()
    return _CACHE["nc"]


def kernel(x, xm, Wy_w, Wy_b, Wz_w, Wz_b, dic, prior, **_unused):
    from concourse.bass_utils import run_bass_kernel_spmd

    nc = _get_nc()
    row = np.ascontiguousarray(np.asarray(dic, dtype=np.float32)[1].reshape(1, C))
    pr = np.ascontiguousarray(np.asarray(prior, dtype=np.float32).reshape(1, 1))
    in_maps = [{"row": row, "prior": pr} for _ in range(N_CORES)]
    res = run_bass_kernel_spmd(nc, in_maps, list(range(N_CORES)))
    shards = [res.results[i]["out"] for i in range(N_CORES)]
    full = np.concatenate(shards, axis=0).reshape(L, 1, C)
    return full
